# revision 46
# baseline (speedup 1.0000x reference)
"""EvoAttn (V-only causal self-attention) on 8 Trainium2 NeuronCores.

Full input x:(2,2048,2048) fp32 -> full output (2,2048,2048) fp32.
Sharding: 32 (b,h) head-slices, 4 per core (head parallel).

Per (b,h) on-device (L=2048, D=128), with V = x[b,:,h*128:(h+1)*128]:
  S^T tiles  : psum[k=128, q<=512] = VT[:,kblk].T @ VT[:,qchunk]   (bf16 PE)
  E^T tiles  : exp(S^T / sqrt(D)); work is split between ScalarE (exact
               spline exp) and DVE (Schraudolph fast exp: t = trunc(s*A+B)
               as int16, bitcast to bf16 == 2^y with linear mantissa
               interpolation; ~3% sawtooth == perturbing s by <=0.03,
               harmless off the diagonal). Diagonal 128-col sub-blocks
               always take the exact path (they dominate softmax).
  causal mask: diagonal 128x128 sub-block multiplied by host mask (Pool)
  PV         : psum[q=128, 129] += E^T[:,qsub].T @ Vaug[kblk]  where Vaug
               has a ones column -> col 128 = softmax denominator
  normalize  : rec = 1/psum[:,128] (DVE); out = psum[:,:128]*rec via
               ScalarE activation-Copy(scale=rec) or DVE, load-balanced
"""

import sys

for _p in ("/opt/trn_rl_repo",):
    if _p not in sys.path:
        sys.path.insert(0, _p)

import numpy as np
import ml_dtypes

BF16 = ml_dtypes.bfloat16

B, L, E = 2, 2048, 2048
H, D = 16, 128
P = 128          # partition dim / k-block
QC = 1024        # q chunk (two PSUM banks of fp32)
NKB = L // P     # 16 k-blocks
NQC = L // QC    # 2 q chunks
QB = QC // P     # 8 q-blocks (PV granularity) per chunk
NCORES = 8
H4 = (B * H) // NCORES  # 4 heads per core
SCALE = 1.0 / float(np.sqrt(D))

# Schraudolph fast-exp constants (bf16 bit space):
# bf16 bits of exp(s*SCALE) ~= trunc(s*FE_A + FE_B); bitcast int16->bf16.
# FE_B = 127*128 - C + 0.5 with C=6.5 calibrated to zero the mean
# multiplicative sawtooth error (max ~4%, mean ~0.2%).
FE_A = SCALE * float(np.log2(np.e)) * 128.0
FE_B = 16256.0 - 6.5 + 0.5

# engine-cost estimates (ns, trace-calibrated) for the norm router
def _act_cost(cols):
    return cols * 0.833 + 195.0


def _dve_cost(cols):
    return cols * 0.85 + 90.0


NORM_ACT = 412.0  # measured activation-Copy(scale) [128,128] cost
NORM_DVE = 302.0  # measured tensor_scalar_mul [128,128] cost

N_WARMUP = 5     # PE warmup matmuls during the DMA prologue (HAM ramp);
                 # must END before the first vt chunk lands (~9us) or the
                 # warmup delays real work on the in-order PE queue
FLOOR = 32       # PV-mm backlog reserve for chain-less windows
BURST = 12       # emit PV mms in bursts of ~this size
BUDGET_DIV = 170  # per-tile PV budget = cols//BUDGET_DIV + 2

_cache = {}


def _build_nc():
    import concourse.bacc as bacc
    import concourse.mybir as mybir
    import concourse.tile as tile
    from contextlib import ExitStack

    f32 = mybir.dt.float32
    bf16 = mybir.dt.bfloat16
    i16 = mybir.dt.int16

    nc = bacc.Bacc("TRN2", target_bir_lowering=False, debug=False,
                   num_devices=NCORES)

    # DRAM I/O (per-core shapes); inputs are chunk-major so every DMA
    # chunk is one fully contiguous DRAM block (maximal packets)
    x_vq = nc.dram_tensor("vq", [H4, 2, P, QB * (D + 1)], bf16,
                          kind="ExternalInput")
    x_vt = nc.dram_tensor("vt", [H4, 4, D, 512], bf16, kind="ExternalInput")
    x_mask = nc.dram_tensor("mask", [P, P], bf16, kind="ExternalInput")
    y = nc.dram_tensor("y", [H4, L, D], f32, kind="ExternalOutput")

    EXP = mybir.ActivationFunctionType.Exp
    COPY = mybir.ActivationFunctionType.Copy
    MULT = mybir.AluOpType.mult
    ADD = mybir.AluOpType.add

    with tile.TileContext(nc) as tc, ExitStack() as ctx:
        const_pool = ctx.enter_context(tc.tile_pool(name="const", bufs=1))
        vq_pool = ctx.enter_context(tc.tile_pool(name="vq", bufs=2))
        vt_pool = ctx.enter_context(tc.tile_pool(name="vt", bufs=2))
        pt_pool = ctx.enter_context(tc.tile_pool(name="pt", bufs=46))
        out_pool = ctx.enter_context(tc.tile_pool(name="out", bufs=2))
        rec_pool = ctx.enter_context(tc.tile_pool(name="rec", bufs=4))
        ps_s = ctx.enter_context(tc.tile_pool(name="ps_s", bufs=3, space="PSUM"))
        ps_o = ctx.enter_context(tc.tile_pool(name="ps_o", bufs=2, space="PSUM"))

        mask_t = const_pool.tile([P, P], bf16)
        # (mask DMA is emitted inside load_head(0), after vt chunk 0, so
        # the first scores tile's input heads the sync queue)

        # zero scratch with no DMA dependency: lets the PE warmup and the
        # exp-table preload start ~1us into the kernel instead of waiting
        # for the first DMA to land (~7us)
        warm_src = const_pool.tile([P, 512], bf16)
        # gpsimd's sequencer reaches "main" earliest (~6.0us vs DVE ~6.9)
        nc.gpsimd.memset(warm_src[:], 0.0)

        # preload the exp activation table while input DMAs run
        warm_sc = rec_pool.tile([P, 1], f32, tag="rec")
        nc.scalar.activation(warm_sc[:], warm_src[:, 0:1], EXP, scale=1.0)

        # PE warmup: keep the PE array busy through the input-DMA prologue
        # so the HAM clock-gate is at full rate when real tiles start
        ps_w = ps_s.tile([P, QC], f32, tag="ps_s")
        for w in range(N_WARMUP):
            nc.tensor.matmul(ps_w[:, 0:512], warm_src[:, 0:P],
                             warm_src[:], start=True, stop=True)

        # running busy estimates for the exp/norm router
        bal = {"act": 0.0, "dve": 0.0}

        # per-head state created lazily by the flat tile stream
        vt_ts, vq_ts, o_bigs, pt_tiles = {}, {}, {}, {}

        def load_head(h):
            vt_t = vt_pool.tile([P, L], bf16, tag="vt")
            vq_t = vq_pool.tile([P, NKB, D + 1], bf16, tag="vq")
            if h == 0:
                # first head: spread across BOTH HWDGE queues (sync+scalar)
                # so the ~650ns/DMA descriptor generations parallelize and
                # the first scores tile (needs vt[:, :1024]) starts asap
                nc.sync.dma_start(vt_t[:, 0:512], x_vt[h][0])
                nc.scalar.dma_start(vt_t[:, 512:1024], x_vt[h][1])
                nc.sync.dma_start(mask_t[:], x_mask[:, :])
                nc.sync.dma_start(vt_t[:, 1024:1536], x_vt[h][2])
                nc.scalar.dma_start(
                    vq_t[:, 0:QB, :],
                    x_vq[h][0].rearrange("p (kb c) -> p kb c", kb=QB))
                nc.sync.dma_start(vt_t[:, 1536:2048], x_vt[h][3])
                nc.scalar.dma_start(
                    vq_t[:, QB:2 * QB, :],
                    x_vq[h][1].rearrange("p (kb c) -> p kb c", kb=QB))
            else:
                for c in range(4):
                    nc.sync.dma_start(vt_t[:, c * 512:(c + 1) * 512],
                                      x_vt[h][c])
                for c in range(2):
                    nc.sync.dma_start(
                        vq_t[:, c * QB:(c + 1) * QB, :],
                        x_vq[h][c].rearrange("p (kb c) -> p kb c", kb=QB))
            vt_ts[h], vq_ts[h] = vt_t, vq_t
            o_bigs[h] = out_pool.tile([P, NKB, D], f32, tag="obig", name="obig")

        def emit_fast_exp(ps, pt, a, b):
            nc.vector.tensor_scalar(
                pt[:, a:b].bitcast(i16), ps[:, a:b],
                FE_A, FE_B, op0=MULT, op1=ADD)

        def emit_scores_tile(h, qc, kb):
            j = kb - QB * qc  # >=0 -> diagonal-chunk block
            off = max(0, j) * P
            vt_t = vt_ts[h]
            q0 = qc * QC
            ps = ps_s.tile([P, QC], f32, tag="ps_s")
            if off < 512:
                nc.tensor.matmul(ps[:, off:512],
                                 vt_t[:, kb * P:(kb + 1) * P],
                                 vt_t[:, q0 + off:q0 + 512],
                                 start=True, stop=True)
                nc.tensor.matmul(ps[:, 512:],
                                 vt_t[:, kb * P:(kb + 1) * P],
                                 vt_t[:, q0 + 512:q0 + QC],
                                 start=True, stop=True)
            else:
                nc.tensor.matmul(ps[:, off:],
                                 vt_t[:, kb * P:(kb + 1) * P],
                                 vt_t[:, q0 + off:q0 + QC],
                                 start=True, stop=True)
            pt = pt_pool.tile([P, QC], bf16, tag="pt")
            if j >= 0:
                # diag-chunk tile: diagonal sub-block needs exact exp on
                # ACT (it dominates softmax) + causal mask on Pool
                # (consumed a few us later by the chain's final PV matmul
                # -> Pool latency hidden). The causal remainder either
                # folds into one combined ACT instruction or goes fast on
                # DVE, by load balance.
                rem = QC - off - P
                mk_a = max(bal["act"] + _act_cost(P + rem), bal["dve"])
                mk_d = max(bal["act"] + _act_cost(P),
                           bal["dve"] + _dve_cost(rem)) if rem else mk_a + 1
                if mk_a <= mk_d or rem == 0:
                    bal["act"] += _act_cost(P + rem)
                    nc.scalar.activation(pt[:, off:], ps[:, off:],
                                         EXP, scale=SCALE)
                else:
                    bal["act"] += _act_cost(P)
                    bal["dve"] += _dve_cost(rem)
                    nc.scalar.activation(pt[:, off:off + P],
                                         ps[:, off:off + P], EXP, scale=SCALE)
                    emit_fast_exp(ps, pt, off + P, QC)
                nc.gpsimd.tensor_mul(pt[:, off:off + P],
                                     pt[:, off:off + P], mask_t[:])
            else:
                # strictly-off-diagonal tile: exact ACT exp or Schraudolph
                # fast exp on DVE, by load balance (fine interleaving of
                # the two engines is essential: long single-engine runs
                # serialize the pipeline)
                mk_a = max(bal["act"] + _act_cost(QC), bal["dve"])
                mk_d = max(bal["act"], bal["dve"] + _dve_cost(QC))
                if mk_a <= mk_d:
                    bal["act"] += _act_cost(QC)
                    nc.scalar.activation(pt[:], ps[:], EXP, scale=SCALE)
                else:
                    bal["dve"] += _dve_cost(QC)
                    emit_fast_exp(ps, pt, 0, QC)
            pt_tiles[(h, qc, kb)] = pt

        # PV chain work is drained as individual matmuls from a FIFO so
        # each scores tile is followed by just enough PV matmuls to keep
        # PE streaming while ACT/DVE run exp. A backlog floor keeps work
        # in reserve for the chain-less pass-1-early windows.
        chain_fifo = []   # (h, qc, qi) in completion order
        cur = {"mm": 0, "po": None}   # cursor into chain_fifo[0]
        backlog = {"mms": 0}

        def finish_block(h, qc, qi):
            po = cur["po"]
            rec = rec_pool.tile([P, 1], f32, tag="rec")
            nc.vector.reciprocal(rec[:], po[:, D:D + 1])
            bal["dve"] += 135.0
            # normalize: out = po[:, :D] * rec ; route by balance
            mk_a = max(bal["act"] + NORM_ACT, bal["dve"])
            mk_d = max(bal["act"], bal["dve"] + NORM_DVE)
            if mk_a <= mk_d:
                bal["act"] += NORM_ACT
                nc.scalar.activation(o_bigs[h][:, qi, :], po[:, :D], COPY,
                                     scale=rec[:])
            else:
                bal["dve"] += NORM_DVE
                nc.vector.tensor_scalar_mul(o_bigs[h][:, qi, :], po[:, :D],
                                            rec[:])
            # output drains ride the HWDGE queues: frees the Pool engine
            # for the causal masks and skips the costly SWDGE descriptor
            # generation + epilogue drain. The last head drains in half-
            # quarters alternating sync/scalar queues so the final
            # transfer is small and its descriptor gen is overlapped.
            if h == H4 - 1:
                # last head: half-quarter drains, and PER-BLOCK for the
                # final two q-blocks so the very last transfer (the
                # serial tail) is only 64KB
                if qi == 14 or qi == 15:
                    eng = nc.scalar if qi == 14 else nc.sync
                    eng.dma_start(
                        y[h][qi * 128:(qi + 1) * 128, :],
                        o_bigs[h][:, qi, :],
                    )
                elif qi % 2 == 1:
                    q2_ = qi // 2
                    eng = nc.scalar if qi % 4 == 1 else nc.sync
                    eng.dma_start(
                        y[h][q2_ * 256:(q2_ + 1) * 256, :].rearrange(
                            "(kb p) d -> p kb d", p=P),
                        o_bigs[h][:, q2_ * 2:(q2_ + 1) * 2, :],
                    )
            elif qi % 4 == 3:  # finished an output quarter -> drain it
                q4 = qi // 4
                nc.sync.dma_start(
                    y[h][q4 * 512:(q4 + 1) * 512, :].rearrange(
                        "(kb p) d -> p kb d", p=P),
                    o_bigs[h][:, q4 * 4:(q4 + 1) * 4, :],
                )

        def emit_chain_mms(n):
            while n > 0 and chain_fifo:
                h, qc, qi = chain_fifo[0]
                qsub = qi - QB * qc
                if cur["po"] is None:
                    cur["po"] = ps_o.tile([P, D + 1], f32, tag="ps_o",
                                          name="po")
                    cur["mm"] = 0
                kb = cur["mm"]
                nc.tensor.matmul(
                    cur["po"][:],
                    pt_tiles[(h, qc, kb)][:, qsub * P:(qsub + 1) * P],
                    vq_ts[h][:, kb, :],
                    start=(kb == 0), stop=(kb == qi),
                )
                cur["mm"] += 1
                backlog["mms"] -= 1
                n -= 1
                if cur["mm"] == qi + 1:
                    finish_block(h, qc, qi)
                    chain_fifo.pop(0)
                    cur["po"] = None

        budget_acc = {"n": 0}
        for h in range(H4):
            load_head(h)
            for qc in range(NQC):
                for kb in range(QB * qc + QB):
                    j = kb - QB * qc
                    # the reserve exists FOR the chain-less pass-1-early
                    # window: release it there (and on the last head),
                    # hold it during append windows
                    # spend the PV reserve EVENLY through the chain-less
                    # qc1-early window (and the final chunk) by ramping
                    # the floor down tile-by-tile: releasing it all at the
                    # window start starves the window's last tiles
                    in_p1_early = qc == NQC - 1 and j < 0
                    last_chunk = h == H4 - 1 and qc == NQC - 1
                    if last_chunk:
                        floor = FLOOR * (NKB - 1 - kb) // (NKB - 1)
                    elif in_p1_early:
                        floor = FLOOR * (QB - 1 - kb) // (QB + 1)
                    else:
                        floor = FLOOR
                    emit_scores_tile(h, qc, kb)
                    if j >= 0:
                        qi = QB * qc + j
                        chain_fifo.append((h, qc, qi))
                        backlog["mms"] += qi + 1
                    cols = QC - max(0, j) * P
                    budget_acc["n"] += cols // BUDGET_DIV + 2
                    if budget_acc["n"] >= BURST:
                        emit_chain_mms(
                            min(budget_acc["n"], backlog["mms"] - floor))
                        budget_acc["n"] = 0
        emit_chain_mms(backlog["mms"])

    nc.compile()
    return nc


def _get_nc():
    if "nc" not in _cache:
        _cache["nc"] = _build_nc()
    return _cache["nc"]


def _make_mask():
    # keep (partition=k_local, free=q_local) where q_local >= k_local
    pk = np.arange(P)[:, None]
    fq = np.arange(P)[None, :]
    return (fq >= pk).astype(BF16)


def kernel(x):
    from concourse.bass_utils import run_bass_kernel_spmd

    x = np.asarray(x)
    in_dtype = x.dtype
    assert x.shape == (B, L, E)

    nc = _get_nc()

    # (B, L, H, D) -> (B*H, L, D), bf16
    v = np.ascontiguousarray(
        x.reshape(B, L, H, D).transpose(0, 2, 1, 3)
    ).reshape(B * H, L, D).astype(BF16)

    mask = _make_mask()
    in_maps = []
    for c in range(NCORES):
        sl = v[H4 * c:H4 * (c + 1)]                      # (H4, L, D)
        # chunk-major vq: [H4, 2, P, QB*(D+1)], ones column appended
        vq = np.ones((H4, P, NKB, D + 1), dtype=BF16)
        vq[..., :D] = sl.reshape(H4, NKB, P, D).transpose(0, 2, 1, 3)
        vq = np.ascontiguousarray(
            vq.reshape(H4, P, 2, QB * (D + 1)).transpose(0, 2, 1, 3))
        # chunk-major vt: [H4, 4, D, 512]
        vt = sl.transpose(0, 2, 1).reshape(H4, D, 4, 512)
        vt = np.ascontiguousarray(vt.transpose(0, 2, 1, 3))
        in_maps.append({"vq": vq, "vt": vt, "mask": mask})

    import os

    kwargs = {}
    if os.environ.get("KERNEL_TRACE"):
        kwargs["trace"] = True
        if os.environ.get("KERNEL_TRACE_DIR"):
            kwargs["tmpdir"] = os.environ["KERNEL_TRACE_DIR"]
    res = run_bass_kernel_spmd(nc, in_maps, core_ids=list(range(NCORES)), **kwargs)
    _cache["last_results"] = res
    ys = np.stack([res.results[c]["y"] for c in range(NCORES)], axis=0)
    # (NCORES, H4, L, D) -> (B, H, L, D) -> (B, L, E)
    out = ys.reshape(B, H, L, D).transpose(0, 2, 1, 3).reshape(B, L, E)
    return out.astype(in_dtype, copy=False)


# revision 47
# speedup vs baseline: 1.0475x; 1.0475x over previous
"""EvoAttn (V-only causal self-attention) on 8 Trainium2 NeuronCores.

Full input x:(2,2048,2048) fp32 -> full output (2,2048,2048) fp32.
Sharding: 32 (b,h) head-slices, 4 per core (head parallel).

Per (b,h) on-device (L=2048, D=128), with V = x[b,:,h*128:(h+1)*128]:
  S^T tiles  : psum[k=128, q<=512] = VT[:,kblk].T @ VT[:,qchunk]   (bf16 PE)
  E^T tiles  : exp(S^T / sqrt(D)); work is split between ScalarE (exact
               spline exp) and DVE (Schraudolph fast exp: t = trunc(s*A+B)
               as int16, bitcast to bf16 == 2^y with linear mantissa
               interpolation; ~3% sawtooth == perturbing s by <=0.03,
               harmless off the diagonal). Diagonal 128-col sub-blocks
               always take the exact path (they dominate softmax).
  causal mask: diagonal 128x128 sub-block multiplied by host mask (Pool)
  PV         : psum[q=128, 129] += E^T[:,qsub].T @ Vaug[kblk]  where Vaug
               has a ones column -> col 128 = softmax denominator
  normalize  : rec = 1/psum[:,128] (DVE); out = psum[:,:128]*rec via
               ScalarE activation-Copy(scale=rec) or DVE, load-balanced
"""

import sys

for _p in ("/opt/trn_rl_repo",):
    if _p not in sys.path:
        sys.path.insert(0, _p)

import numpy as np
import ml_dtypes

BF16 = ml_dtypes.bfloat16

B, L, E = 2, 2048, 2048
H, D = 16, 128
P = 128          # partition dim / k-block
QC = 1024        # q chunk (two PSUM banks of fp32)
NKB = L // P     # 16 k-blocks
NQC = L // QC    # 2 q chunks
QB = QC // P     # 8 q-blocks (PV granularity) per chunk
NCORES = 8
H4 = (B * H) // NCORES  # 4 heads per core
SCALE = 1.0 / float(np.sqrt(D))

# Schraudolph fast-exp constants (bf16 bit space):
# bf16 bits of exp(s*SCALE) ~= trunc(s*FE_A + FE_B); bitcast int16->bf16.
# FE_B = 127*128 - C + 0.5 with C=6.5 calibrated to zero the mean
# multiplicative sawtooth error (max ~4%, mean ~0.2%).
FE_A = SCALE * float(np.log2(np.e)) * 128.0
FE_B = 16256.0 - 6.5 + 0.5

# engine-cost estimates (ns, trace-calibrated) for the norm router
def _act_cost(cols):
    return cols * 0.833 + 195.0


def _dve_cost(cols):
    return cols * 0.85 + 90.0


NORM_ACT = 412.0  # measured activation-Copy(scale) [128,128] cost
NORM_DVE = 302.0  # measured tensor_scalar_mul [128,128] cost

N_WARMUP = 5     # PE warmup matmuls during the DMA prologue (HAM ramp);
                 # must END before the first vt chunk lands (~9us) or the
                 # warmup delays real work on the in-order PE queue
FLOOR = 32       # PV-mm backlog reserve for chain-less windows
BURST = 12       # emit PV mms in bursts of ~this size
BUDGET_DIV = 170  # per-tile PV budget = cols//BUDGET_DIV + 2

_cache = {}


def _build_nc():
    import concourse.bacc as bacc
    import concourse.mybir as mybir
    import concourse.tile as tile
    from contextlib import ExitStack

    f32 = mybir.dt.float32
    bf16 = mybir.dt.bfloat16
    i16 = mybir.dt.int16

    nc = bacc.Bacc("TRN2", target_bir_lowering=False, debug=False,
                   num_devices=NCORES)

    # DRAM I/O (per-core shapes); inputs are chunk-major so every DMA
    # chunk is one fully contiguous DRAM block (maximal packets)
    x_vq = nc.dram_tensor("vq", [H4, 2, P, QB * (D + 1)], bf16,
                          kind="ExternalInput")
    x_vt = nc.dram_tensor("vt", [H4, 4, D, 512], bf16, kind="ExternalInput")
    x_mask = nc.dram_tensor("mask", [P, P], bf16, kind="ExternalInput")
    y = nc.dram_tensor("y", [H4, L, D], f32, kind="ExternalOutput")

    EXP = mybir.ActivationFunctionType.Exp
    COPY = mybir.ActivationFunctionType.Copy
    MULT = mybir.AluOpType.mult
    ADD = mybir.AluOpType.add

    with tile.TileContext(nc) as tc, ExitStack() as ctx:
        const_pool = ctx.enter_context(tc.tile_pool(name="const", bufs=1))
        vq_pool = ctx.enter_context(tc.tile_pool(name="vq", bufs=2))
        vt_pool = ctx.enter_context(tc.tile_pool(name="vt", bufs=2))
        pt_pool = ctx.enter_context(tc.tile_pool(name="pt", bufs=40))
        out_pool = ctx.enter_context(tc.tile_pool(name="out", bufs=2))
        rec_pool = ctx.enter_context(tc.tile_pool(name="rec", bufs=4))
        ps_s = ctx.enter_context(tc.tile_pool(name="ps_s", bufs=3, space="PSUM"))
        ps_o = ctx.enter_context(tc.tile_pool(name="ps_o", bufs=2, space="PSUM"))

        mask_t = const_pool.tile([P, P], bf16)
        # (mask DMA is emitted inside load_head(0), after vt chunk 0, so
        # the first scores tile's input heads the sync queue)

        # zero scratch with no DMA dependency: lets the PE warmup and the
        # exp-table preload start ~1us into the kernel instead of waiting
        # for the first DMA to land (~7us)
        warm_src = const_pool.tile([P, 512], bf16)
        # gpsimd's sequencer reaches "main" earliest (~6.0us vs DVE ~6.9)
        nc.gpsimd.memset(warm_src[:], 0.0)

        # preload the exp activation table while input DMAs run
        warm_sc = rec_pool.tile([P, 1], f32, tag="rec")
        nc.scalar.activation(warm_sc[:], warm_src[:, 0:1], EXP, scale=1.0)

        # PE warmup: keep the PE array busy through the input-DMA prologue
        # so the HAM clock-gate is at full rate when real tiles start
        ps_w = ps_s.tile([P, QC], f32, tag="ps_s")
        for w in range(N_WARMUP):
            nc.tensor.matmul(ps_w[:, 0:512], warm_src[:, 0:P],
                             warm_src[:], start=True, stop=True)

        # running busy estimates for the exp/norm router
        bal = {"act": 0.0, "dve": 0.0}

        # per-head state created lazily by the flat tile stream
        vt_ts, vq_ts, o_bigs, pt_tiles = {}, {}, {}, {}

        def load_head(h):
            vt_t = vt_pool.tile([P, L], bf16, tag="vt")
            vq_t = vq_pool.tile([P, NKB, D + 1], bf16, tag="vq")
            if h == 0:
                # first head: spread across BOTH HWDGE queues (sync+scalar)
                # so the ~650ns/DMA descriptor generations parallelize and
                # the first scores tile (needs vt[:, :1024]) starts asap
                nc.sync.dma_start(vt_t[:, 0:512], x_vt[h][0])
                nc.scalar.dma_start(vt_t[:, 512:1024], x_vt[h][1])
                nc.sync.dma_start(mask_t[:], x_mask[:, :])
                nc.sync.dma_start(vt_t[:, 1024:1536], x_vt[h][2])
                nc.scalar.dma_start(
                    vq_t[:, 0:QB, :],
                    x_vq[h][0].rearrange("p (kb c) -> p kb c", kb=QB))
                nc.sync.dma_start(vt_t[:, 1536:2048], x_vt[h][3])
                nc.scalar.dma_start(
                    vq_t[:, QB:2 * QB, :],
                    x_vq[h][1].rearrange("p (kb c) -> p kb c", kb=QB))
            else:
                for c in range(4):
                    nc.sync.dma_start(vt_t[:, c * 512:(c + 1) * 512],
                                      x_vt[h][c])
                for c in range(2):
                    nc.sync.dma_start(
                        vq_t[:, c * QB:(c + 1) * QB, :],
                        x_vq[h][c].rearrange("p (kb c) -> p kb c", kb=QB))
            vt_ts[h], vq_ts[h] = vt_t, vq_t
            o_bigs[h] = out_pool.tile([P, NKB, D], f32, tag="obig", name="obig")

        def emit_fast_exp(ps, pt, a, b):
            nc.vector.tensor_scalar(
                pt[:, a:b].bitcast(i16), ps[:, a:b],
                FE_A, FE_B, op0=MULT, op1=ADD)

        def emit_scores_tile(h, qc, kb):
            j = kb - QB * qc  # >=0 -> diagonal-chunk block
            off = max(0, j) * P
            vt_t = vt_ts[h]
            q0 = qc * QC
            ps = ps_s.tile([P, QC], f32, tag="ps_s")
            if off < 512:
                nc.tensor.matmul(ps[:, off:512],
                                 vt_t[:, kb * P:(kb + 1) * P],
                                 vt_t[:, q0 + off:q0 + 512],
                                 start=True, stop=True)
                nc.tensor.matmul(ps[:, 512:],
                                 vt_t[:, kb * P:(kb + 1) * P],
                                 vt_t[:, q0 + 512:q0 + QC],
                                 start=True, stop=True)
            else:
                nc.tensor.matmul(ps[:, off:],
                                 vt_t[:, kb * P:(kb + 1) * P],
                                 vt_t[:, q0 + off:q0 + QC],
                                 start=True, stop=True)
            pt = pt_pool.tile([P, QC], bf16, tag="pt")
            if j >= 0:
                # diag-chunk tile: diagonal sub-block needs exact exp on
                # ACT (it dominates softmax) + causal mask on Pool
                # (consumed a few us later by the chain's final PV matmul
                # -> Pool latency hidden). The causal remainder either
                # folds into one combined ACT instruction or goes fast on
                # DVE, by load balance.
                rem = QC - off - P
                mk_a = max(bal["act"] + _act_cost(P + rem), bal["dve"])
                mk_d = max(bal["act"] + _act_cost(P),
                           bal["dve"] + _dve_cost(rem)) if rem else mk_a + 1
                if mk_a <= mk_d or rem == 0:
                    bal["act"] += _act_cost(P + rem)
                    nc.scalar.activation(pt[:, off:], ps[:, off:],
                                         EXP, scale=SCALE)
                else:
                    bal["act"] += _act_cost(P)
                    bal["dve"] += _dve_cost(rem)
                    nc.scalar.activation(pt[:, off:off + P],
                                         ps[:, off:off + P], EXP, scale=SCALE)
                    emit_fast_exp(ps, pt, off + P, QC)
                nc.gpsimd.tensor_mul(pt[:, off:off + P],
                                     pt[:, off:off + P], mask_t[:])
            else:
                # strictly-off-diagonal tile: exact ACT exp or Schraudolph
                # fast exp on DVE, by load balance (fine interleaving of
                # the two engines is essential: long single-engine runs
                # serialize the pipeline)
                mk_a = max(bal["act"] + _act_cost(QC), bal["dve"])
                mk_d = max(bal["act"], bal["dve"] + _dve_cost(QC))
                if mk_a <= mk_d:
                    bal["act"] += _act_cost(QC)
                    nc.scalar.activation(pt[:], ps[:], EXP, scale=SCALE)
                else:
                    bal["dve"] += _dve_cost(QC)
                    emit_fast_exp(ps, pt, 0, QC)
            pt_tiles[(h, qc, kb)] = pt

        # PV chain work is drained as individual matmuls from a FIFO so
        # each scores tile is followed by just enough PV matmuls to keep
        # PE streaming while ACT/DVE run exp. A backlog floor keeps work
        # in reserve for the chain-less pass-1-early windows.
        chain_fifo = []   # (h, qc, qi) in completion order
        cur = {"mm": 0, "po": None}   # cursor into chain_fifo[0]
        backlog = {"mms": 0}

        def finish_block(h, qc, qi):
            po = cur["po"]
            rec = rec_pool.tile([P, 1], f32, tag="rec")
            nc.vector.reciprocal(rec[:], po[:, D:D + 1])
            bal["dve"] += 135.0
            # normalize: out = po[:, :D] * rec ; route by balance
            mk_a = max(bal["act"] + NORM_ACT, bal["dve"])
            mk_d = max(bal["act"], bal["dve"] + NORM_DVE)
            if mk_a <= mk_d:
                bal["act"] += NORM_ACT
                nc.scalar.activation(o_bigs[h][:, qi, :], po[:, :D], COPY,
                                     scale=rec[:])
            else:
                bal["dve"] += NORM_DVE
                nc.vector.tensor_scalar_mul(o_bigs[h][:, qi, :], po[:, :D],
                                            rec[:])
            # output drains ride the HWDGE queues: frees the Pool engine
            # for the causal masks and skips the costly SWDGE descriptor
            # generation + epilogue drain. The last head drains in half-
            # quarters alternating sync/scalar queues so the final
            # transfer is small and its descriptor gen is overlapped.
            if h == H4 - 1:
                # last head: half-quarter drains, and PER-BLOCK for the
                # final two q-blocks so the very last transfer (the
                # serial tail) is only 64KB
                if qi == 14 or qi == 15:
                    eng = nc.scalar if qi == 14 else nc.sync
                    eng.dma_start(
                        y[h][qi * 128:(qi + 1) * 128, :],
                        o_bigs[h][:, qi, :],
                    )
                elif qi % 2 == 1:
                    q2_ = qi // 2
                    eng = nc.scalar if qi % 4 == 1 else nc.sync
                    eng.dma_start(
                        y[h][q2_ * 256:(q2_ + 1) * 256, :].rearrange(
                            "(kb p) d -> p kb d", p=P),
                        o_bigs[h][:, q2_ * 2:(q2_ + 1) * 2, :],
                    )
            elif qi % 4 == 3:  # finished an output quarter -> drain it
                q4 = qi // 4
                nc.sync.dma_start(
                    y[h][q4 * 512:(q4 + 1) * 512, :].rearrange(
                        "(kb p) d -> p kb d", p=P),
                    o_bigs[h][:, q4 * 4:(q4 + 1) * 4, :],
                )

        def emit_chain_mms(n):
            while n > 0 and chain_fifo:
                h, qc, qi = chain_fifo[0]
                qsub = qi - QB * qc
                if cur["po"] is None:
                    cur["po"] = ps_o.tile([P, D + 1], f32, tag="ps_o",
                                          name="po")
                    cur["mm"] = 0
                kb = cur["mm"]
                nc.tensor.matmul(
                    cur["po"][:],
                    pt_tiles[(h, qc, kb)][:, qsub * P:(qsub + 1) * P],
                    vq_ts[h][:, kb, :],
                    start=(kb == 0), stop=(kb == qi),
                )
                cur["mm"] += 1
                backlog["mms"] -= 1
                n -= 1
                if cur["mm"] == qi + 1:
                    finish_block(h, qc, qi)
                    chain_fifo.pop(0)
                    cur["po"] = None

        budget_acc = {"n": 0}
        for h in range(H4):
            load_head(h)
            for qc in range(NQC):
                for kb in range(QB * qc + QB):
                    j = kb - QB * qc
                    # the reserve exists FOR the chain-less pass-1-early
                    # window: release it there (and on the last head),
                    # hold it during append windows
                    # spend the PV reserve EVENLY through the chain-less
                    # qc1-early window (and the final chunk) by ramping
                    # the floor down tile-by-tile: releasing it all at the
                    # window start starves the window's last tiles
                    in_p1_early = qc == NQC - 1 and j < 0
                    last_chunk = h == H4 - 1 and qc == NQC - 1
                    if last_chunk:
                        floor = FLOOR * (NKB - 1 - kb) // (NKB - 1)
                    elif in_p1_early:
                        floor = FLOOR * (QB - 1 - kb) // (QB + 1)
                    else:
                        floor = FLOOR
                    emit_scores_tile(h, qc, kb)
                    if j >= 0:
                        qi = QB * qc + j
                        chain_fifo.append((h, qc, qi))
                        backlog["mms"] += qi + 1
                    cols = QC - max(0, j) * P
                    budget_acc["n"] += cols // BUDGET_DIV + 2
                    if budget_acc["n"] >= BURST:
                        emit_chain_mms(
                            min(budget_acc["n"], backlog["mms"] - floor))
                        budget_acc["n"] = 0
        emit_chain_mms(backlog["mms"])

    nc.compile()
    return nc


def _get_nc():
    if "nc" not in _cache:
        _cache["nc"] = _build_nc()
    return _cache["nc"]


def _make_mask():
    # keep (partition=k_local, free=q_local) where q_local >= k_local
    pk = np.arange(P)[:, None]
    fq = np.arange(P)[None, :]
    return (fq >= pk).astype(BF16)


def kernel(x):
    from concourse.bass_utils import run_bass_kernel_spmd

    x = np.asarray(x)
    in_dtype = x.dtype
    assert x.shape == (B, L, E)

    nc = _get_nc()

    # (B, L, H, D) -> (B*H, L, D), bf16
    v = np.ascontiguousarray(
        x.reshape(B, L, H, D).transpose(0, 2, 1, 3)
    ).reshape(B * H, L, D).astype(BF16)

    mask = _make_mask()
    in_maps = []
    for c in range(NCORES):
        sl = v[H4 * c:H4 * (c + 1)]                      # (H4, L, D)
        # chunk-major vq: [H4, 2, P, QB*(D+1)], ones column appended
        vq = np.ones((H4, P, NKB, D + 1), dtype=BF16)
        vq[..., :D] = sl.reshape(H4, NKB, P, D).transpose(0, 2, 1, 3)
        vq = np.ascontiguousarray(
            vq.reshape(H4, P, 2, QB * (D + 1)).transpose(0, 2, 1, 3))
        # chunk-major vt: [H4, 4, D, 512]
        vt = sl.transpose(0, 2, 1).reshape(H4, D, 4, 512)
        vt = np.ascontiguousarray(vt.transpose(0, 2, 1, 3))
        in_maps.append({"vq": vq, "vt": vt, "mask": mask})

    import os

    kwargs = {}
    if os.environ.get("KERNEL_TRACE"):
        kwargs["trace"] = True
        if os.environ.get("KERNEL_TRACE_DIR"):
            kwargs["tmpdir"] = os.environ["KERNEL_TRACE_DIR"]
    res = run_bass_kernel_spmd(nc, in_maps, core_ids=list(range(NCORES)), **kwargs)
    _cache["last_results"] = res
    ys = np.stack([res.results[c]["y"] for c in range(NCORES)], axis=0)
    # (NCORES, H4, L, D) -> (B, H, L, D) -> (B, L, E)
    out = ys.reshape(B, H, L, D).transpose(0, 2, 1, 3).reshape(B, L, E)
    return out.astype(in_dtype, copy=False)


# revision 48
# speedup vs baseline: 1.0717x; 1.0231x over previous
"""EvoAttn (V-only causal self-attention) on 8 Trainium2 NeuronCores.

Full input x:(2,2048,2048) fp32 -> full output (2,2048,2048) fp32.
Sharding: 32 (b,h) head-slices, 4 per core (head parallel).

Per (b,h) on-device (L=2048, D=128), with V = x[b,:,h*128:(h+1)*128]:
  S^T tiles  : psum[k=128, q<=512] = VT[:,kblk].T @ VT[:,qchunk]   (bf16 PE)
  E^T tiles  : exp(S^T / sqrt(D)); work is split between ScalarE (exact
               spline exp) and DVE (Schraudolph fast exp: t = trunc(s*A+B)
               as int16, bitcast to bf16 == 2^y with linear mantissa
               interpolation; ~3% sawtooth == perturbing s by <=0.03,
               harmless off the diagonal). Diagonal 128-col sub-blocks
               always take the exact path (they dominate softmax).
  causal mask: diagonal 128x128 sub-block multiplied by host mask (Pool)
  PV         : psum[q=128, 129] += E^T[:,qsub].T @ Vaug[kblk]  where Vaug
               has a ones column -> col 128 = softmax denominator
  normalize  : rec = 1/psum[:,128] (DVE); out = psum[:,:128]*rec via
               ScalarE activation-Copy(scale=rec) or DVE, load-balanced
"""

import sys

for _p in ("/opt/trn_rl_repo",):
    if _p not in sys.path:
        sys.path.insert(0, _p)

import numpy as np
import ml_dtypes

BF16 = ml_dtypes.bfloat16

B, L, E = 2, 2048, 2048
H, D = 16, 128
P = 128          # partition dim / k-block
QC = 1024        # q chunk (two PSUM banks of fp32)
NKB = L // P     # 16 k-blocks
NQC = L // QC    # 2 q chunks
QB = QC // P     # 8 q-blocks (PV granularity) per chunk
NCORES = 8
H4 = (B * H) // NCORES  # 4 heads per core
SCALE = 1.0 / float(np.sqrt(D))

# Schraudolph fast-exp constants (bf16 bit space):
# bf16 bits of exp(s*SCALE) ~= trunc(s*FE_A + FE_B); bitcast int16->bf16.
# FE_B = 127*128 - C + 0.5 with C=6.5 calibrated to zero the mean
# multiplicative sawtooth error (max ~4%, mean ~0.2%).
FE_A = SCALE * float(np.log2(np.e)) * 128.0
FE_B = 16256.0 - 6.5 + 0.5

# engine-cost estimates (ns, trace-calibrated) for the norm router
def _act_cost(cols):
    return cols * 0.833 + 195.0


def _dve_cost(cols):
    return cols * 0.85 + 90.0


NORM_ACT = 412.0  # measured activation-Copy(scale) [128,128] cost
NORM_DVE = 302.0  # measured tensor_scalar_mul [128,128] cost

N_WARMUP = 5     # PE warmup matmuls during the DMA prologue (HAM ramp);
                 # must END before the first vt chunk lands (~9us) or the
                 # warmup delays real work on the in-order PE queue
FLOOR = 32       # PV-mm backlog reserve for chain-less windows
BURST = 12       # emit PV mms in bursts of ~this size
BUDGET_DIV = 150  # per-tile PV budget = cols//BUDGET_DIV + 2

_cache = {}


def _build_nc():
    import concourse.bacc as bacc
    import concourse.mybir as mybir
    import concourse.tile as tile
    from contextlib import ExitStack

    f32 = mybir.dt.float32
    bf16 = mybir.dt.bfloat16
    i16 = mybir.dt.int16

    nc = bacc.Bacc("TRN2", target_bir_lowering=False, debug=False,
                   num_devices=NCORES)

    # DRAM I/O (per-core shapes); inputs are chunk-major so every DMA
    # chunk is one fully contiguous DRAM block (maximal packets)
    x_vq = nc.dram_tensor("vq", [H4, 2, P, QB * (D + 1)], bf16,
                          kind="ExternalInput")
    x_vt = nc.dram_tensor("vt", [H4, 4, D, 512], bf16, kind="ExternalInput")
    x_mask = nc.dram_tensor("mask", [P, P], bf16, kind="ExternalInput")
    y = nc.dram_tensor("y", [H4, L, D], f32, kind="ExternalOutput")

    EXP = mybir.ActivationFunctionType.Exp
    COPY = mybir.ActivationFunctionType.Copy
    MULT = mybir.AluOpType.mult
    ADD = mybir.AluOpType.add

    with tile.TileContext(nc) as tc, ExitStack() as ctx:
        const_pool = ctx.enter_context(tc.tile_pool(name="const", bufs=1))
        vq_pool = ctx.enter_context(tc.tile_pool(name="vq", bufs=2))
        vt_pool = ctx.enter_context(tc.tile_pool(name="vt", bufs=2))
        pt_pool = ctx.enter_context(tc.tile_pool(name="pt", bufs=40))
        out_pool = ctx.enter_context(tc.tile_pool(name="out", bufs=2))
        rec_pool = ctx.enter_context(tc.tile_pool(name="rec", bufs=4))
        ps_s = ctx.enter_context(tc.tile_pool(name="ps_s", bufs=3, space="PSUM"))
        ps_o = ctx.enter_context(tc.tile_pool(name="ps_o", bufs=2, space="PSUM"))

        mask_t = const_pool.tile([P, P], bf16)
        # (mask DMA is emitted inside load_head(0), after vt chunk 0, so
        # the first scores tile's input heads the sync queue)

        # zero scratch with no DMA dependency: lets the PE warmup and the
        # exp-table preload start ~1us into the kernel instead of waiting
        # for the first DMA to land (~7us)
        warm_src = const_pool.tile([P, 512], bf16)
        # gpsimd's sequencer reaches "main" earliest (~6.0us vs DVE ~6.9)
        nc.gpsimd.memset(warm_src[:], 0.0)

        # preload the exp activation table while input DMAs run
        warm_sc = rec_pool.tile([P, 1], f32, tag="rec")
        nc.scalar.activation(warm_sc[:], warm_src[:, 0:1], EXP, scale=1.0)

        # PE warmup: keep the PE array busy through the input-DMA prologue
        # so the HAM clock-gate is at full rate when real tiles start
        ps_w = ps_s.tile([P, QC], f32, tag="ps_s")
        for w in range(N_WARMUP):
            nc.tensor.matmul(ps_w[:, 0:512], warm_src[:, 0:P],
                             warm_src[:], start=True, stop=True)

        # running busy estimates for the exp/norm router
        bal = {"act": 0.0, "dve": 0.0}

        # per-head state created lazily by the flat tile stream
        vt_ts, vq_ts, o_bigs, pt_tiles = {}, {}, {}, {}

        def load_head(h):
            vt_t = vt_pool.tile([P, L], bf16, tag="vt")
            vq_t = vq_pool.tile([P, NKB, D + 1], bf16, tag="vq")
            if h == 0:
                # first head: spread across BOTH HWDGE queues (sync+scalar)
                # so the ~650ns/DMA descriptor generations parallelize and
                # the first scores tile (needs vt[:, :1024]) starts asap
                nc.sync.dma_start(vt_t[:, 0:512], x_vt[h][0])
                nc.scalar.dma_start(vt_t[:, 512:1024], x_vt[h][1])
                nc.sync.dma_start(mask_t[:], x_mask[:, :])
                nc.sync.dma_start(vt_t[:, 1024:1536], x_vt[h][2])
                nc.scalar.dma_start(
                    vq_t[:, 0:QB, :],
                    x_vq[h][0].rearrange("p (kb c) -> p kb c", kb=QB))
                nc.sync.dma_start(vt_t[:, 1536:2048], x_vt[h][3])
                nc.scalar.dma_start(
                    vq_t[:, QB:2 * QB, :],
                    x_vq[h][1].rearrange("p (kb c) -> p kb c", kb=QB))
            else:
                for c in range(4):
                    nc.sync.dma_start(vt_t[:, c * 512:(c + 1) * 512],
                                      x_vt[h][c])
                for c in range(2):
                    nc.sync.dma_start(
                        vq_t[:, c * QB:(c + 1) * QB, :],
                        x_vq[h][c].rearrange("p (kb c) -> p kb c", kb=QB))
            vt_ts[h], vq_ts[h] = vt_t, vq_t
            o_bigs[h] = out_pool.tile([P, NKB, D], f32, tag="obig", name="obig")

        def emit_fast_exp(ps, pt, a, b):
            nc.vector.tensor_scalar(
                pt[:, a:b].bitcast(i16), ps[:, a:b],
                FE_A, FE_B, op0=MULT, op1=ADD)

        def emit_scores_tile(h, qc, kb):
            j = kb - QB * qc  # >=0 -> diagonal-chunk block
            off = max(0, j) * P
            vt_t = vt_ts[h]
            q0 = qc * QC
            ps = ps_s.tile([P, QC], f32, tag="ps_s")
            if off < 512:
                nc.tensor.matmul(ps[:, off:512],
                                 vt_t[:, kb * P:(kb + 1) * P],
                                 vt_t[:, q0 + off:q0 + 512],
                                 start=True, stop=True)
                nc.tensor.matmul(ps[:, 512:],
                                 vt_t[:, kb * P:(kb + 1) * P],
                                 vt_t[:, q0 + 512:q0 + QC],
                                 start=True, stop=True)
            else:
                nc.tensor.matmul(ps[:, off:],
                                 vt_t[:, kb * P:(kb + 1) * P],
                                 vt_t[:, q0 + off:q0 + QC],
                                 start=True, stop=True)
            pt = pt_pool.tile([P, QC], bf16, tag="pt")
            if j >= 0:
                # diag-chunk tile: diagonal sub-block needs exact exp on
                # ACT (it dominates softmax) + causal mask on Pool
                # (consumed a few us later by the chain's final PV matmul
                # -> Pool latency hidden). The causal remainder either
                # folds into one combined ACT instruction or goes fast on
                # DVE, by load balance.
                rem = QC - off - P
                mk_a = max(bal["act"] + _act_cost(P + rem), bal["dve"])
                mk_d = max(bal["act"] + _act_cost(P),
                           bal["dve"] + _dve_cost(rem)) if rem else mk_a + 1
                if mk_a <= mk_d or rem == 0:
                    bal["act"] += _act_cost(P + rem)
                    nc.scalar.activation(pt[:, off:], ps[:, off:],
                                         EXP, scale=SCALE)
                else:
                    bal["act"] += _act_cost(P)
                    bal["dve"] += _dve_cost(rem)
                    nc.scalar.activation(pt[:, off:off + P],
                                         ps[:, off:off + P], EXP, scale=SCALE)
                    emit_fast_exp(ps, pt, off + P, QC)
                nc.gpsimd.tensor_mul(pt[:, off:off + P],
                                     pt[:, off:off + P], mask_t[:])
            else:
                # strictly-off-diagonal tile: exact ACT exp or Schraudolph
                # fast exp on DVE, by load balance (fine interleaving of
                # the two engines is essential: long single-engine runs
                # serialize the pipeline)
                mk_a = max(bal["act"] + _act_cost(QC), bal["dve"])
                mk_d = max(bal["act"], bal["dve"] + _dve_cost(QC))
                if mk_a <= mk_d:
                    bal["act"] += _act_cost(QC)
                    nc.scalar.activation(pt[:], ps[:], EXP, scale=SCALE)
                else:
                    bal["dve"] += _dve_cost(QC)
                    emit_fast_exp(ps, pt, 0, QC)
            pt_tiles[(h, qc, kb)] = pt

        # PV chain work is drained as individual matmuls from a FIFO so
        # each scores tile is followed by just enough PV matmuls to keep
        # PE streaming while ACT/DVE run exp. A backlog floor keeps work
        # in reserve for the chain-less pass-1-early windows.
        chain_fifo = []   # (h, qc, qi) in completion order
        cur = {"mm": 0, "po": None}   # cursor into chain_fifo[0]
        backlog = {"mms": 0}

        def finish_block(h, qc, qi):
            po = cur["po"]
            rec = rec_pool.tile([P, 1], f32, tag="rec")
            nc.vector.reciprocal(rec[:], po[:, D:D + 1])
            bal["dve"] += 135.0
            # normalize: out = po[:, :D] * rec ; route by balance
            mk_a = max(bal["act"] + NORM_ACT, bal["dve"])
            mk_d = max(bal["act"], bal["dve"] + NORM_DVE)
            if mk_a <= mk_d:
                bal["act"] += NORM_ACT
                nc.scalar.activation(o_bigs[h][:, qi, :], po[:, :D], COPY,
                                     scale=rec[:])
            else:
                bal["dve"] += NORM_DVE
                nc.vector.tensor_scalar_mul(o_bigs[h][:, qi, :], po[:, :D],
                                            rec[:])
            # output drains ride the HWDGE queues: frees the Pool engine
            # for the causal masks and skips the costly SWDGE descriptor
            # generation + epilogue drain. The last head drains in half-
            # quarters alternating sync/scalar queues so the final
            # transfer is small and its descriptor gen is overlapped.
            if h == H4 - 1:
                # last head: half-quarter drains, and PER-BLOCK for the
                # final two q-blocks so the very last transfer (the
                # serial tail) is only 64KB
                if qi == 14 or qi == 15:
                    eng = nc.scalar if qi == 14 else nc.sync
                    eng.dma_start(
                        y[h][qi * 128:(qi + 1) * 128, :],
                        o_bigs[h][:, qi, :],
                    )
                elif qi % 2 == 1:
                    q2_ = qi // 2
                    eng = nc.scalar if qi % 4 == 1 else nc.sync
                    eng.dma_start(
                        y[h][q2_ * 256:(q2_ + 1) * 256, :].rearrange(
                            "(kb p) d -> p kb d", p=P),
                        o_bigs[h][:, q2_ * 2:(q2_ + 1) * 2, :],
                    )
            elif qi % 4 == 3:  # finished an output quarter -> drain it
                q4 = qi // 4
                nc.sync.dma_start(
                    y[h][q4 * 512:(q4 + 1) * 512, :].rearrange(
                        "(kb p) d -> p kb d", p=P),
                    o_bigs[h][:, q4 * 4:(q4 + 1) * 4, :],
                )

        def emit_chain_mms(n):
            while n > 0 and chain_fifo:
                h, qc, qi = chain_fifo[0]
                qsub = qi - QB * qc
                if cur["po"] is None:
                    cur["po"] = ps_o.tile([P, D + 1], f32, tag="ps_o",
                                          name="po")
                    cur["mm"] = 0
                kb = cur["mm"]
                nc.tensor.matmul(
                    cur["po"][:],
                    pt_tiles[(h, qc, kb)][:, qsub * P:(qsub + 1) * P],
                    vq_ts[h][:, kb, :],
                    start=(kb == 0), stop=(kb == qi),
                )
                cur["mm"] += 1
                backlog["mms"] -= 1
                n -= 1
                if cur["mm"] == qi + 1:
                    finish_block(h, qc, qi)
                    chain_fifo.pop(0)
                    cur["po"] = None

        budget_acc = {"n": 0}
        for h in range(H4):
            load_head(h)
            for qc in range(NQC):
                for kb in range(QB * qc + QB):
                    j = kb - QB * qc
                    # the reserve exists FOR the chain-less pass-1-early
                    # window: release it there (and on the last head),
                    # hold it during append windows
                    # spend the PV reserve EVENLY through the chain-less
                    # qc1-early window (and the final chunk) by ramping
                    # the floor down tile-by-tile: releasing it all at the
                    # window start starves the window's last tiles
                    in_p1_early = qc == NQC - 1 and j < 0
                    last_chunk = h == H4 - 1 and qc == NQC - 1
                    if last_chunk:
                        floor = FLOOR * (NKB - 1 - kb) // (NKB - 1)
                    elif in_p1_early:
                        floor = FLOOR * (QB - 1 - kb) // (QB + 1)
                    else:
                        floor = FLOOR
                    emit_scores_tile(h, qc, kb)
                    if j >= 0:
                        qi = QB * qc + j
                        chain_fifo.append((h, qc, qi))
                        backlog["mms"] += qi + 1
                    cols = QC - max(0, j) * P
                    budget_acc["n"] += cols // BUDGET_DIV + 2
                    if budget_acc["n"] >= BURST:
                        emit_chain_mms(
                            min(budget_acc["n"], backlog["mms"] - floor))
                        budget_acc["n"] = 0
        emit_chain_mms(backlog["mms"])

    nc.compile()
    return nc


def _get_nc():
    if "nc" not in _cache:
        _cache["nc"] = _build_nc()
    return _cache["nc"]


def _make_mask():
    # keep (partition=k_local, free=q_local) where q_local >= k_local
    pk = np.arange(P)[:, None]
    fq = np.arange(P)[None, :]
    return (fq >= pk).astype(BF16)


def kernel(x):
    from concourse.bass_utils import run_bass_kernel_spmd

    x = np.asarray(x)
    in_dtype = x.dtype
    assert x.shape == (B, L, E)

    nc = _get_nc()

    # (B, L, H, D) -> (B*H, L, D), bf16
    v = np.ascontiguousarray(
        x.reshape(B, L, H, D).transpose(0, 2, 1, 3)
    ).reshape(B * H, L, D).astype(BF16)

    mask = _make_mask()
    in_maps = []
    for c in range(NCORES):
        sl = v[H4 * c:H4 * (c + 1)]                      # (H4, L, D)
        # chunk-major vq: [H4, 2, P, QB*(D+1)], ones column appended
        vq = np.ones((H4, P, NKB, D + 1), dtype=BF16)
        vq[..., :D] = sl.reshape(H4, NKB, P, D).transpose(0, 2, 1, 3)
        vq = np.ascontiguousarray(
            vq.reshape(H4, P, 2, QB * (D + 1)).transpose(0, 2, 1, 3))
        # chunk-major vt: [H4, 4, D, 512]
        vt = sl.transpose(0, 2, 1).reshape(H4, D, 4, 512)
        vt = np.ascontiguousarray(vt.transpose(0, 2, 1, 3))
        in_maps.append({"vq": vq, "vt": vt, "mask": mask})

    import os

    kwargs = {}
    if os.environ.get("KERNEL_TRACE"):
        kwargs["trace"] = True
        if os.environ.get("KERNEL_TRACE_DIR"):
            kwargs["tmpdir"] = os.environ["KERNEL_TRACE_DIR"]
    res = run_bass_kernel_spmd(nc, in_maps, core_ids=list(range(NCORES)), **kwargs)
    _cache["last_results"] = res
    ys = np.stack([res.results[c]["y"] for c in range(NCORES)], axis=0)
    # (NCORES, H4, L, D) -> (B, H, L, D) -> (B, L, E)
    out = ys.reshape(B, H, L, D).transpose(0, 2, 1, 3).reshape(B, L, E)
    return out.astype(in_dtype, copy=False)
